# revision 1
# baseline (speedup 1.0000x reference)
"""HBiLSTM Trainium2 kernel.

Strategy (8 NeuronCores):
  - cores 0-3: forward LSTM + fwd highway half, 8 samples each
  - cores 4-7: backward LSTM on host-reversed input + bwd highway half
  All cores run the SAME SPMD program; direction is encoded purely in the
  per-core input data (weights + pre-reversed/pre-transposed x).

Device layout is "transposed" (layout T): hidden/gate dims on SBUF
partitions, batch on the free dim.  Host does all transposes / reversal /
concat / masking (untimed).

Phases on device, per core (8 samples, T=512, DIN=512, H=256):
  A: xg.T = Wp @ x.T + b  (Wp = [Wih(perm); Wg_half]  -> 10 gate tiles of 128)
  B: 512-step LSTM recurrence, Whh.T stationary (bf16, FWL), 2 interleaved
     chains of 4 samples to hide the per-step dependency-chain latency.
  C: highway gate flow = g_pre + sig(g_pre) * (y - g_pre), bulk, then DMA out.
"""

import numpy as np
import ml_dtypes

bf16 = ml_dtypes.bfloat16

B, T, DIN, H = 32, 512, 512, 256
NG = 4 * H          # 1024 gate rows per direction
NP = NG + H         # 1280 = gates + highway-half rows
BPC = 8             # samples per core
NCORES = 8
TOK = BPC * T       # tokens per core = 4096

# gate reorder: torch order i,f,g,o -> i,f,o,g  (so sigmoid gates are tiles 0:6,
# tanh gate is tiles 6:8 when viewed as 8 tiles of 128)
_PERM = np.concatenate([np.arange(0, 512), np.arange(768, 1024), np.arange(512, 768)])

_PROG_CACHE = {}


def _build_program(n_steps=T, static=True, unroll=16, nchain=2):
    import concourse.bacc as bacc
    import concourse.mybir as mybir
    import concourse.tile as tile
    import concourse.bass as bass

    fp32 = mybir.dt.float32
    b16 = mybir.dt.bfloat16

    nc = bacc.Bacc(None)

    xt_d = nc.dram_tensor("xt", [DIN, TOK], b16, kind="ExternalInput")
    wpt_d = nc.dram_tensor("wpt", [DIN, NP], b16, kind="ExternalInput")
    whht_d = nc.dram_tensor("whht", [H, NG], b16, kind="ExternalInput")
    bias_d = nc.dram_tensor("bias", [NP], fp32, kind="ExternalInput")
    out_d = nc.dram_tensor("out", [128, 2, T, BPC], fp32, kind="ExternalOutput")

    KT_A = DIN // 128      # 4 contraction tiles in phase A
    MT_A = NP // 128       # 10 output tiles in phase A (8 xg + 2 gpre)
    NCH_A = TOK // 512     # 8 token chunks of 512
    GT = NG // 128         # 8 gate tiles in recurrence
    KT_B = H // 128        # 2 contraction tiles in recurrence
    NCHAIN = nchain
    CB = BPC // NCHAIN

    with tile.TileContext(nc) as tc:
      with (
          tc.tile_pool(name="persist", bufs=1) as pp,
          tc.tile_pool(name="psum", bufs=2, space="PSUM") as psp,
      ):
        gpre = pp.tile([128, 2, T, BPC], fp32, tag="gpre")      # 32KB/p
        bias_sb = pp.tile([128, MT_A], fp32, tag="bias")
        nc.sync.dma_start(bias_sb[:], bias_d.rearrange("(m p) -> p m", p=128))

        whh_sb = pp.tile([128, KT_B, NG], b16, tag="whh")
        nc.sync.dma_start(whh_sb[:], whht_d.rearrange("(k p) m -> p k m", p=128))

        yh = [
            pp.tile([128, KT_B, n_steps + 1, CB], b16, tag=f"yh{ch}", name=f"yh{ch}")
            for ch in range(NCHAIN)
        ]
        cst = [
            pp.tile([128, KT_B, 1, CB], fp32, tag=f"c{ch}", name=f"c{ch}")
            for ch in range(NCHAIN)
        ]
        for ch in range(NCHAIN):
            nc.gpsimd.memset(yh[ch][:, :, 0, :], 0.0)
            nc.gpsimd.memset(cst[ch][:], 0.0)

        with tc.tile_pool(name="pxg", bufs=1) as pxg:
            xg = pxg.tile([128, GT, T, BPC], fp32, tag="xg")    # 128KB/p

            # ---------------- Phase A: projections ----------------
            with tc.tile_pool(name="phaseA", bufs=2) as pa:
                wp_sb = pa.tile([128, KT_A, NP], b16, tag="wp", bufs=1)
                nc.sync.dma_start(
                    wp_sb[:], wpt_d.rearrange("(k p) m -> p k m", p=128)
                )
                for n in range(NCH_A):
                    xt_sb = pa.tile([128, KT_A, 512], b16, tag="xt")
                    nc.sync.dma_start(
                        xt_sb[:],
                        xt_d.rearrange("(k p) n -> p k n", p=128)[
                            :, :, n * 512 : (n + 1) * 512
                        ],
                    )
                    for m in range(MT_A):
                        ps = psp.tile([128, 512], fp32, tag="psA", bufs=2)
                        for k in range(KT_A):
                            nc.tensor.matmul(
                                ps[:],
                                wp_sb[:, k, m * 128 : (m + 1) * 128],
                                xt_sb[:, k, :],
                                start=(k == 0),
                                stop=(k == KT_A - 1),
                            )
                        tchunk = ps[:].rearrange("p (t b) -> p t b", b=BPC)
                        t0 = n * (512 // BPC)
                        t1 = (n + 1) * (512 // BPC)
                        if m < GT:
                            dst = xg[:, m, t0:t1, :]
                        else:
                            dst = gpre[:, m - GT, t0:t1, :]
                        nc.vector.tensor_scalar_add(dst, tchunk, bias_sb[:, m : m + 1])

            # ---------------- Phase B: recurrence ----------------
            with tc.tile_pool(name="phaseB", bufs=6) as pb:

                def step(t):
                    if static:
                        tsl = lambda off: slice(t + off, t + off + 1)
                    else:
                        tsl = lambda off: bass.ds(t + off, 1)
                    for ch in range(NCHAIN):
                        cb = ch * CB
                        ps = psp.tile(
                            [128, GT, 1, CB], fp32, tag=f"psB{ch}", bufs=3, name=f"psB{ch}"
                        )
                        for m in range(GT):
                            for k in range(KT_B):
                                nc.tensor.matmul(
                                    ps[:, m, :, :],
                                    whh_sb[:, k, m * 128 : (m + 1) * 128],
                                    yh[ch][:, k, tsl(0), :],
                                    start=(k == 0),
                                    stop=(k == KT_B - 1),
                                )
                        gf = pb.tile([128, GT, 1, CB], fp32, tag=f"gf{ch}", name=f"gf{ch}")
                        nc.vector.tensor_add(
                            gf[:], ps[:], xg[:, :, tsl(0), cb : cb + CB]
                        )
                        sig = pb.tile([128, 6, 1, CB], fp32, tag=f"sig{ch}", name=f"sig{ch}")
                        nc.scalar.activation(
                            sig[:], gf[:, 0:6, :, :],
                            mybir.ActivationFunctionType.Sigmoid,
                        )
                        tgg = pb.tile([128, 2, 1, CB], fp32, tag=f"tg{ch}", name=f"tg{ch}")
                        nc.scalar.activation(
                            tgg[:], gf[:, 6:8, :, :],
                            mybir.ActivationFunctionType.Tanh,
                        )
                        t1_ = pb.tile([128, 2, 1, CB], fp32, tag=f"t1{ch}", name=f"t1{ch}")
                        nc.vector.tensor_mul(t1_[:], sig[:, 0:2, :, :], tgg[:])
                        t2_ = pb.tile([128, 2, 1, CB], fp32, tag=f"t2{ch}", name=f"t2{ch}")
                        nc.vector.tensor_mul(t2_[:], sig[:, 2:4, :, :], cst[ch][:])
                        nc.vector.tensor_add(cst[ch][:], t1_[:], t2_[:])
                        tau = pb.tile([128, 2, 1, CB], fp32, tag=f"tau{ch}", name=f"tau{ch}")
                        nc.scalar.activation(
                            tau[:], cst[ch][:], mybir.ActivationFunctionType.Tanh,
                        )
                        nc.vector.tensor_mul(
                            yh[ch][:, :, tsl(1), :],
                            sig[:, 4:6, :, :],
                            tau[:],
                        )

                if static:
                    for t in range(n_steps):
                        step(t)
                else:
                    tc.For_i_unrolled(0, n_steps, 1, step, max_unroll=unroll)

        # ---------------- Phase C: highway gate ----------------
        with tc.tile_pool(name="phaseC", bufs=2) as pc:
            TC = 128
            for cch in range(T // TC):
                t0, t1 = cch * TC, (cch + 1) * TC
                gp = gpre[:, :, t0:t1, :]
                tg = pc.tile([128, 2, TC, BPC], fp32, tag="tg_c")
                nc.scalar.activation(tg[:], gp, mybir.ActivationFunctionType.Sigmoid)
                yc = pc.tile([128, 2, TC, BPC], fp32, tag="y_c")
                for ch in range(NCHAIN):
                    cb = ch * CB
                    nc.vector.tensor_sub(
                        yc[:, :, :, cb : cb + CB],
                        yh[ch][:, :, t0 + 1 : t1 + 1, :],
                        gp[:, :, :, cb : cb + CB],
                    )
                fl = pc.tile([128, 2, TC, BPC], fp32, tag="fl_c")
                nc.vector.tensor_mul(fl[:], tg[:], yc[:])
                nc.vector.tensor_add(fl[:], fl[:], gp)
                nc.sync.dma_start(out_d[:, :, t0:t1, :], fl[:])

    nc.compile()
    return nc


def _reverse_padded_np(x, lens):
    t = np.arange(T)
    idx = np.where(t[None, :] < lens[:, None], lens[:, None] - 1 - t[None, :], t[None, :])
    return np.take_along_axis(x, idx[:, :, None], axis=1), idx


def kernel(x, Wih_f, Whh_f, bih_f, bhh_f, Wih_b, Whh_b, bih_b, bhh_b, Wg, bg,
           x_lengths, **_unused):
    from concourse.bass_utils import run_bass_kernel_spmd

    x = np.asarray(x, dtype=np.float32)
    lens = np.asarray(x_lengths).astype(np.int64)

    xr, idx = _reverse_padded_np(x, lens)

    def dir_weights(Wih, Whh, bih, bhh, wg_half, bg_half):
        Wp = np.concatenate([np.asarray(Wih)[_PERM], wg_half], axis=0)  # [1280, 512]
        wpt = np.ascontiguousarray(Wp.T).astype(bf16)                   # [512, 1280]
        whht = np.ascontiguousarray(np.asarray(Whh)[_PERM].T).astype(bf16)  # [256,1024]
        bias = np.concatenate(
            [(np.asarray(bih) + np.asarray(bhh))[_PERM], bg_half]
        ).astype(np.float32)
        return wpt, whht, bias

    Wg = np.asarray(Wg); bg = np.asarray(bg)
    fw = dir_weights(Wih_f, Whh_f, bih_f, bhh_f, Wg[0:H], bg[0:H])
    bw = dir_weights(Wih_b, Whh_b, bih_b, bhh_b, Wg[H:2*H], bg[H:2*H])

    in_maps = []
    for c in range(NCORES):
        fwd = c < 4
        s0 = (c % 4) * BPC
        xsrc = x if fwd else xr
        xt = np.ascontiguousarray(
            xsrc[s0 : s0 + BPC].transpose(2, 1, 0).reshape(DIN, TOK)
        ).astype(bf16)
        wpt, whht, bias = fw if fwd else bw
        in_maps.append({"xt": xt, "wpt": wpt, "whht": whht, "bias": bias})

    if "prog" not in _PROG_CACHE:
        _PROG_CACHE["prog"] = _build_program()
    nc = _PROG_CACHE["prog"]
    _PROG_CACHE["last_inmaps"] = in_maps

    res = run_bass_kernel_spmd(nc, in_maps, core_ids=list(range(NCORES)))

    full = np.zeros((B, T, 2 * H), dtype=np.float32)
    for c in range(NCORES):
        arr = np.asarray(res.results[c]["out"], dtype=np.float32)  # [128,2,T,BPC]
        half = arr.transpose(3, 2, 1, 0).reshape(BPC, T, H)
        s0 = (c % 4) * BPC
        if c < 4:
            full[s0 : s0 + BPC, :, 0:H] = half
        else:
            # un-reverse within valid lengths
            half = np.take_along_axis(half, idx[s0 : s0 + BPC][:, :, None], axis=1)
            full[s0 : s0 + BPC, :, H : 2 * H] = half

    mask = (np.arange(T)[None, :] < lens[:, None])[:, :, None]
    full *= mask
    return full



# revision 4
# speedup vs baseline: 1.3592x; 1.3592x over previous
"""HBiLSTM Trainium2 kernel (v2).

Strategy (8 NeuronCores):
  - cores 0-3: forward LSTM + fwd highway half, 8 samples each
  - cores 4-7: backward LSTM on host-reversed input + bwd highway half
  All cores run the SAME SPMD program; direction is encoded purely in the
  per-core input data (weights + pre-reversed/pre-transposed x).

Device layout: gate/hidden dims on SBUF partitions, batch on the free dim.
Host does all transposes / reversal / concat / masking (untimed).

v2 recurrence-step optimizations (per chain, 2 chains of 4 samples):
  - xg(t) is injected into PSUM with an identity matmul accumulating into
    the Whh@h gate pre-activations -> no DVE add, ACT reads PSUM directly.
  - tanh half-angle trick: i,f,o rows of all weights/biases pre-scaled by
    0.5 on host, so ONE tanh covers all 8 gate tiles
    (sigmoid(a) = (tanh(a/2)+1)/2).
  - scaled states c^ = 2c, h^ = 2h make the cell update 4 fused
    scalar_tensor_tensor ops:
        A  = (th_f + 1) * c^            # 2 sig_f * c^
        B  = (th_i + 1) * th_g          # 2 sig_i * g
        c^' = 0.5*A + B                 # = 2 c_new
        tau = tanh(0.5 * c^')           # ACT free scale
        h^' = (th_o + 1) * tau          # = 2 h_new
    The extra 0.5 for h^ as matmul input is folded into Whh on host; the
    0.5 for the output is folded into phase C's (h^*0.5 - gpre) op.

Phases:
  A: xg.T = Wp @ x.T + b  (Wp = [Wih(perm,scaled); Wg_half] -> 10 tiles)
  B: 512-step LSTM recurrence, 2 interleaved chains of 4 samples.
  C: highway gate flow = g_pre + sig(g_pre) * (h^/2 - g_pre), bulk DMA out.
"""

import numpy as np
import ml_dtypes

bf16 = ml_dtypes.bfloat16

B, T, DIN, H = 32, 512, 512, 256
NG = 4 * H          # 1024 gate rows per direction
NP = NG + H         # 1280 = gates + highway-half rows
BPC = 8             # samples per core
NCORES = 8
TOK = BPC * T       # tokens per core = 4096

# gate reorder: torch order i,f,g,o -> i,f,o,g  (tiles 0:2=i, 2:4=f, 4:6=o,
# 6:8=g when viewed as 8 tiles of 128)
_PERM = np.concatenate([np.arange(0, 512), np.arange(768, 1024), np.arange(512, 768)])

_PROG_CACHE = {}


def _build_program(n_steps=T):
    import concourse.bacc as bacc
    import concourse.mybir as mybir
    import concourse.tile as tile

    fp32 = mybir.dt.float32
    b16 = mybir.dt.bfloat16
    Tanh = mybir.ActivationFunctionType.Tanh
    Sigmoid = mybir.ActivationFunctionType.Sigmoid
    ADD = mybir.AluOpType.add
    MULT = mybir.AluOpType.mult
    SUB = mybir.AluOpType.subtract

    nc = bacc.Bacc(None)

    xt_d = nc.dram_tensor("xt", [DIN, TOK], b16, kind="ExternalInput")
    wpt_d = nc.dram_tensor("wpt", [DIN, NP], b16, kind="ExternalInput")
    whht_d = nc.dram_tensor("whht", [H, NG], b16, kind="ExternalInput")
    bias_d = nc.dram_tensor("bias", [NP], fp32, kind="ExternalInput")
    ident_d = nc.dram_tensor("ident", [128, 128], b16, kind="ExternalInput")
    out_d = nc.dram_tensor("out", [128, 2, T, BPC], fp32, kind="ExternalOutput")

    KT_A = DIN // 128      # 4 contraction tiles in phase A
    MT_A = NP // 128       # 10 output tiles in phase A (8 xg + 2 gpre)
    NCH_A = TOK // 512     # 8 token chunks of 512
    GT = NG // 128         # 8 gate tiles in recurrence
    KT_B = H // 128        # 2 contraction tiles in recurrence
    NCHAIN = 2
    CB = BPC // NCHAIN     # 4 samples per chain

    with tile.TileContext(nc) as tc:
      with (
          tc.tile_pool(name="persist", bufs=1) as pp,
          tc.tile_pool(name="psum", bufs=2, space="PSUM") as psp,
      ):
        gpre = pp.tile([128, 2, T, BPC], fp32, tag="gpre")      # 32KB/p
        bias_sb = pp.tile([128, MT_A], fp32, tag="bias")
        nc.sync.dma_start(bias_sb[:], bias_d.rearrange("(m p) -> p m", p=128))

        whh_sb = pp.tile([128, KT_B, NG], b16, tag="whh")
        nc.sync.dma_start(whh_sb[:], whht_d.rearrange("(k p) m -> p k m", p=128))

        ident_sb = pp.tile([128, 128], b16, tag="ident")
        nc.sync.dma_start(ident_sb[:], ident_d[:, :])

        yh = [
            pp.tile([128, KT_B, n_steps + 1, CB], b16, tag=f"yh{ch}", name=f"yh{ch}")
            for ch in range(NCHAIN)
        ]
        cst = [
            pp.tile([128, KT_B, CB], fp32, tag=f"c{ch}", name=f"c{ch}")
            for ch in range(NCHAIN)
        ]
        for ch in range(NCHAIN):
            nc.gpsimd.memset(yh[ch][:, :, 0, :], 0.0)
            nc.gpsimd.memset(cst[ch][:], 0.0)

        with tc.tile_pool(name="pxg", bufs=1) as pxg:
            # per-chain xg, laid out so the per-step slice [:, ch, t, :, :]
            # is a fully contiguous [128, GT, CB] identity-matmul rhs
            xgc = pxg.tile([128, NCHAIN, T, GT, CB], b16, tag="xgc")  # 64KB/p

            # ---------------- Phase A: projections ----------------
            with tc.tile_pool(name="phaseA", bufs=2) as pa:
                wp_sb = pa.tile([128, KT_A, NP], b16, tag="wp", bufs=1)
                nc.sync.dma_start(
                    wp_sb[:], wpt_d.rearrange("(k p) m -> p k m", p=128)
                )
                TBC = 512 // BPC   # 64 timesteps per chunk
                for n in range(NCH_A):
                    xt_sb = pa.tile([128, KT_A, 512], b16, tag="xt")
                    nc.sync.dma_start(
                        xt_sb[:],
                        xt_d.rearrange("(k p) n -> p k n", p=128)[
                            :, :, n * 512 : (n + 1) * 512
                        ],
                    )
                    for m in range(MT_A):
                        ps = psp.tile([128, 512], fp32, tag="psA", bufs=2)
                        for k in range(KT_A):
                            nc.tensor.matmul(
                                ps[:],
                                wp_sb[:, k, m * 128 : (m + 1) * 128],
                                xt_sb[:, k, :],
                                start=(k == 0),
                                stop=(k == KT_A - 1),
                            )
                        tchunk = ps[:].rearrange("p (t b) -> p t b", b=BPC)
                        t0 = n * TBC
                        t1 = (n + 1) * TBC
                        if m < GT:
                            for ch in range(NCHAIN):
                                cb = ch * CB
                                nc.vector.tensor_scalar_add(
                                    xgc[:, ch, t0:t1, m, :],
                                    tchunk[:, :, cb : cb + CB],
                                    bias_sb[:, m : m + 1],
                                )
                        else:
                            nc.vector.tensor_scalar_add(
                                gpre[:, m - GT, t0:t1, :],
                                tchunk,
                                bias_sb[:, m : m + 1],
                            )

            # ---------------- Phase B: recurrence ----------------
            with tc.tile_pool(name="phaseB", bufs=6) as pb:
                for t in range(n_steps):
                    for ch in range(NCHAIN):
                        ps = psp.tile(
                            [128, GT, CB], fp32, tag=f"psB{ch}", bufs=3,
                            name=f"psB{ch}",
                        )
                        # xg(t) into PSUM (clears bank), then Whh@h accum
                        nc.tensor.matmul(
                            ps[:], ident_sb[:], xgc[:, ch, t, :, :],
                            start=True, stop=False,
                        )
                        for m in range(GT):
                            for k in range(KT_B):
                                nc.tensor.matmul(
                                    ps[:, m, :],
                                    whh_sb[:, k, m * 128 : (m + 1) * 128],
                                    yh[ch][:, k, t, :],
                                    start=False,
                                    stop=(m == GT - 1 and k == KT_B - 1),
                                )
                        th = pb.tile([128, GT, CB], fp32, tag=f"th{ch}", name=f"th{ch}")
                        nc.scalar.activation(th[:], ps[:], Tanh)
                        # A = (th_f + 1) * c^   (= 2 sig_f c^)
                        A = pb.tile([128, KT_B, CB], fp32, tag=f"A{ch}", name=f"A{ch}")
                        nc.vector.scalar_tensor_tensor(
                            A[:], th[:, 2:4, :], 1.0, cst[ch][:], ADD, MULT
                        )
                        # B = (th_i + 1) * th_g (= 2 sig_i g)
                        Bt = pb.tile([128, KT_B, CB], fp32, tag=f"B{ch}", name=f"B{ch}")
                        nc.vector.scalar_tensor_tensor(
                            Bt[:], th[:, 0:2, :], 1.0, th[:, 6:8, :], ADD, MULT
                        )
                        # c^' = 0.5*A + B  (= 2 c_new)
                        nc.vector.scalar_tensor_tensor(
                            cst[ch][:], A[:], 0.5, Bt[:], MULT, ADD
                        )
                        # tau = tanh(c^' / 2) = tanh(c_new)
                        tau = pb.tile([128, KT_B, CB], fp32, tag=f"tau{ch}", name=f"tau{ch}")
                        nc.scalar.activation(tau[:], cst[ch][:], Tanh, scale=0.5)
                        # h^' = (th_o + 1) * tau (= 2 h_new)
                        nc.vector.scalar_tensor_tensor(
                            yh[ch][:, :, t + 1, :], th[:, 4:6, :], 1.0, tau[:],
                            ADD, MULT,
                        )

        # ---------------- Phase C: highway gate ----------------
        with tc.tile_pool(name="phaseC", bufs=2) as pc:
            TC = 128
            for cch in range(T // TC):
                t0, t1 = cch * TC, (cch + 1) * TC
                gp = gpre[:, :, t0:t1, :]
                tg = pc.tile([128, 2, TC, BPC], fp32, tag="tg_c")
                nc.scalar.activation(tg[:], gp, Sigmoid)
                yc = pc.tile([128, 2, TC, BPC], fp32, tag="y_c")
                for ch in range(NCHAIN):
                    cb = ch * CB
                    for kk in range(KT_B):
                        # yc = h^/2 - gpre   (3D APs: TensorScalarPtr limit)
                        nc.vector.scalar_tensor_tensor(
                            yc[:, kk, :, cb : cb + CB],
                            yh[ch][:, kk, t0 + 1 : t1 + 1, :],
                            0.5,
                            gp[:, kk, :, cb : cb + CB],
                            MULT, SUB,
                        )
                fl = pc.tile([128, 2, TC, BPC], fp32, tag="fl_c")
                nc.vector.tensor_mul(fl[:], tg[:], yc[:])
                nc.vector.tensor_add(fl[:], fl[:], gp)
                nc.sync.dma_start(out_d[:, :, t0:t1, :], fl[:])

    nc.compile()
    return nc


def _reverse_padded_np(x, lens):
    t = np.arange(T)
    idx = np.where(t[None, :] < lens[:, None], lens[:, None] - 1 - t[None, :], t[None, :])
    return np.take_along_axis(x, idx[:, :, None], axis=1), idx


def kernel(x, Wih_f, Whh_f, bih_f, bhh_f, Wih_b, Whh_b, bih_b, bhh_b, Wg, bg,
           x_lengths, **_unused):
    from concourse.bass_utils import run_bass_kernel_spmd

    x = np.asarray(x, dtype=np.float32)
    lens = np.asarray(x_lengths).astype(np.int64)

    xr, idx = _reverse_padded_np(x, lens)

    # row scaling for the tanh half-angle trick: i,f,o gate rows (the first
    # 768 after PERM) get 0.5; g rows 1.0; highway rows 1.0
    rs = np.ones((NP, 1), dtype=np.float64)
    rs[0:768] = 0.5

    def dir_weights(Wih, Whh, bih, bhh, wg_half, bg_half):
        Wp = np.concatenate([np.asarray(Wih)[_PERM], wg_half], axis=0)  # [1280, 512]
        Wp = Wp * rs
        wpt = np.ascontiguousarray(Wp.T).astype(bf16)                   # [512, 1280]
        # Whh gets the row scaling AND a 0.5 for the h^ = 2h input
        Whh_s = np.asarray(Whh)[_PERM] * rs[0:NG] * 0.5
        whht = np.ascontiguousarray(Whh_s.T).astype(bf16)               # [256, 1024]
        bias = np.concatenate(
            [(np.asarray(bih) + np.asarray(bhh))[_PERM], bg_half]
        ) * rs[:, 0]
        return wpt, whht, bias.astype(np.float32)

    Wg = np.asarray(Wg); bg = np.asarray(bg)
    fw = dir_weights(Wih_f, Whh_f, bih_f, bhh_f, Wg[0:H], bg[0:H])
    bw = dir_weights(Wih_b, Whh_b, bih_b, bhh_b, Wg[H:2*H], bg[H:2*H])

    ident = np.eye(128, dtype=bf16)

    in_maps = []
    for c in range(NCORES):
        fwd = c < 4
        s0 = (c % 4) * BPC
        xsrc = x if fwd else xr
        xt = np.ascontiguousarray(
            xsrc[s0 : s0 + BPC].transpose(2, 1, 0).reshape(DIN, TOK)
        ).astype(bf16)
        wpt, whht, bias = fw if fwd else bw
        in_maps.append({"xt": xt, "wpt": wpt, "whht": whht, "bias": bias,
                        "ident": ident})

    if "prog" not in _PROG_CACHE:
        _PROG_CACHE["prog"] = _build_program()
    nc = _PROG_CACHE["prog"]
    _PROG_CACHE["last_inmaps"] = in_maps

    res = run_bass_kernel_spmd(nc, in_maps, core_ids=list(range(NCORES)))

    full = np.zeros((B, T, 2 * H), dtype=np.float32)
    for c in range(NCORES):
        arr = np.asarray(res.results[c]["out"], dtype=np.float32)  # [128,2,T,BPC]
        half = arr.transpose(3, 2, 1, 0).reshape(BPC, T, H)
        s0 = (c % 4) * BPC
        if c < 4:
            full[s0 : s0 + BPC, :, 0:H] = half
        else:
            # un-reverse within valid lengths
            half = np.take_along_axis(half, idx[s0 : s0 + BPC][:, :, None], axis=1)
            full[s0 : s0 + BPC, :, H : 2 * H] = half

    mask = (np.arange(T)[None, :] < lens[:, None])[:, :, None]
    full *= mask
    return full
